# revision 1
# baseline (speedup 1.0000x reference)
"""Trainium2 Bass kernel for the 2-module Mamba-style SSM block.

Sharding: 8 cores = 4 batches x 2 modules (core c -> batch c//2, module c%2).
Each core computes one full branch for one batch; the aggregate+out_proj
matmuls are folded (host-side) into one matrix M_k per module, partial outputs
are pair-wise AllReduced on device, and the host picks one core per batch.

Device layout: channels on partitions, time on the free axis.
The selective scan runs as 256 `tensor_tensor_scan` recurrences per core
(16 channel-tiles x 16 state dims), chunked along L with per-(et,n) carries.
"""
from contextlib import ExitStack

import numpy as np

import concourse.bass as bass
import concourse.tile as tile
from concourse import bacc, mybir
from concourse.bass_utils import run_bass_kernel_spmd

FP = mybir.dt.float32
AX = mybir.AxisListType
OP = mybir.AluOpType
AF = mybir.ActivationFunctionType

B, L, D = 4, 2048, 1024
E, N, CW, K, R = 2048, 16, 4, 2, 64
ET = E // 128          # 16 channel tiles
DT = D // 128          # 8 d_model tiles
LC = 256               # cube chunk length along L
NLC = L // LC          # 8 chunks
MMF = 512              # max matmul moving free size
NCORES = 8
GATE_AF = AF.Silu   # sim_test swaps to Tanh (Silu missing in CoreSim)

_CACHE = {}


def _build_program(dbg=False):
    nc = bacc.Bacc("TRN2", target_bir_lowering=False, debug=False,
                   num_devices=NCORES)

    def din(name, shape):
        return nc.dram_tensor(name, list(shape), FP, kind="ExternalInput").ap()

    hsT = din("hsT", (D, L))          # hidden_states[b].T
    winT = din("winT", (D, 2 * E))    # in_proj_w.T
    xpT = din("xpT", (E, R + 2 * N))  # x_proj_w.T
    dtwT = din("dtwT", (R, E))        # dt_w[k].T
    dtb = din("dtb", (E, 1))
    convw = din("convw", (E, CW))
    convb = din("convb", (E, 1))
    Amat = din("Amat", (E, N))        # -exp(A_log[k])
    Dp = din("Dp", (E, 1))
    Mk = din("Mk", (E, D))            # (out_w @ agg_w[:, k*E:(k+1)*E]).T
    out = nc.dram_tensor("out", [L, D], FP, kind="ExternalOutput").ap()

    kind = dict(kind="ExternalOutput") if dbg else {}
    xspill = nc.dram_tensor("xspill", [E, L], FP, **kind).ap()
    zspill = nc.dram_tensor("zspill", [E, L], FP, **kind).ap()
    bcspill = nc.dram_tensor("bcspill", [2 * N, L], FP, **kind).ap()
    yspill = nc.dram_tensor("yspill", [E, L], FP, **kind).ap() if dbg else None

    with tile.TileContext(nc) as tc, ExitStack() as ctx:
        const = ctx.enter_context(tc.tile_pool(name="const", bufs=1))
        dram = ctx.enter_context(tc.tile_pool(name="dram", bufs=1, space="DRAM"))
        bigw_pool = ctx.enter_context(tc.tile_pool(name="bigw", bufs=1))
        wpool = ctx.enter_context(tc.tile_pool(name="wpool", bufs=2))
        ppool = ctx.enter_context(tc.tile_pool(name="ppool", bufs=2, space="PSUM"))
        sm_pool = ctx.enter_context(tc.tile_pool(name="sm", bufs=2))
        smf_pool = ctx.enter_context(tc.tile_pool(name="smf", bufs=2))
        nu_pool = ctx.enter_context(tc.tile_pool(name="nu", bufs=3))
        big_pool = ctx.enter_context(tc.tile_pool(name="big", bufs=3))
        bc_pool = ctx.enter_context(tc.tile_pool(name="bc", bufs=1))
        po_pool = ctx.enter_context(tc.tile_pool(name="po", bufs=1, space="PSUM"))

        opart = dram.tile([L, D], FP)
        oshared = dram.tile([L, D], FP)

        # ---- resident constants ----
        hs_sb = bigw_pool.tile([128, DT, L], FP, tag="bigw")  # 8 MB, P1 only
        for dt_ in range(DT):
            nc.sync.dma_start(out=hs_sb[:, dt_, :],
                              in_=hsT[dt_ * 128:(dt_ + 1) * 128, :])
        xpT_sb = const.tile([128, ET, R + 2 * N], FP)
        nc.sync.dma_start(out=xpT_sb,
                          in_=xpT.rearrange("(a p) c -> p a c", p=128))
        dtwT_sb = const.tile([R, ET, 128], FP)
        nc.sync.dma_start(out=dtwT_sb,
                          in_=dtwT.rearrange("p (a c) -> p a c", c=128))
        Amat_sb = const.tile([128, ET, N], FP)
        nc.sync.dma_start(out=Amat_sb,
                          in_=Amat.rearrange("(a p) n -> p a n", p=128))
        dtb_sb = const.tile([128, ET, 1], FP)
        nc.sync.dma_start(out=dtb_sb, in_=dtb.rearrange("(a p) o -> p a o", p=128))
        Dp_sb = const.tile([128, ET, 1], FP)
        nc.sync.dma_start(out=Dp_sb, in_=Dp.rearrange("(a p) o -> p a o", p=128))
        convw_sb = const.tile([128, ET, CW], FP)
        nc.sync.dma_start(out=convw_sb,
                          in_=convw.rearrange("(a p) c -> p a c", p=128))
        convb_sb = const.tile([128, ET, 1], FP)
        nc.sync.dma_start(out=convb_sb,
                          in_=convb.rearrange("(a p) o -> p a o", p=128))
        carry = const.tile([128, ET, N], FP)

        # ---- phase 1: in_proj + depthwise conv + silu; spill x and silu(z) ----
        for ct in range(2 * ET):
            win_ct = wpool.tile([128, DT, 128], FP, tag="win")
            nc.sync.dma_start(
                out=win_ct,
                in_=winT[:, ct * 128:(ct + 1) * 128].rearrange(
                    "(a p) c -> p a c", p=128))
            if ct < ET:
                xpad = big_pool.tile([128, L + CW - 1], FP, tag="cube")
                nc.vector.memset(xpad[:, 0:CW - 1], 0.0)
            for ls in range(L // MMF):
                psum = ppool.tile([128, MMF], FP, tag="mm")
                for dt_ in range(DT):
                    nc.tensor.matmul(psum, win_ct[:, dt_, :],
                                     hs_sb[:, dt_, ls * MMF:(ls + 1) * MMF],
                                     start=(dt_ == 0), stop=(dt_ == DT - 1))
                if ct < ET:
                    nc.scalar.activation(
                        out=xpad[:, CW - 1 + ls * MMF:CW - 1 + (ls + 1) * MMF],
                        in_=psum, func=AF.Copy)
                else:
                    zt = smf_pool.tile([128, MMF], FP, tag="zt")
                    nc.scalar.activation(out=zt, in_=psum, func=GATE_AF)
                    nc.sync.dma_start(
                        out=zspill[(ct - ET) * 128:(ct - ET + 1) * 128,
                                   ls * MMF:(ls + 1) * MMF],
                        in_=zt)
            if ct < ET:
                acc = big_pool.tile([128, L], FP, tag="cube")
                nc.vector.tensor_scalar(out=acc, in0=xpad[:, 0:L],
                                        scalar1=convw_sb[:, ct, 0:1],
                                        scalar2=None, op0=OP.mult)
                for j in range(1, CW):
                    nc.vector.scalar_tensor_tensor(
                        out=acc, in0=xpad[:, j:j + L],
                        scalar=convw_sb[:, ct, j:j + 1], in1=acc,
                        op0=OP.mult, op1=OP.add)
                nc.scalar.activation(out=acc, in_=acc, func=GATE_AF,
                                     bias=convb_sb[:, ct, :], scale=1.0)
                nc.sync.dma_start(out=xspill[ct * 128:(ct + 1) * 128, :], in_=acc)

        # ---- phase 2: x_proj -> x_dbl (96, L) resident ----
        xdbl_sb = const.tile([R + 2 * N, L], FP)
        for ls in range(L // MMF):
            psx = ppool.tile([R + 2 * N, MMF], FP, tag="mm")
            for et in range(ET):
                xl = smf_pool.tile([128, MMF], FP, tag="xl2")
                nc.sync.dma_start(
                    out=xl, in_=xspill[et * 128:(et + 1) * 128,
                                       ls * MMF:(ls + 1) * MMF])
                nc.tensor.matmul(psx, xpT_sb[:, et, :], xl,
                                 start=(et == 0), stop=(et == ET - 1))
            nc.scalar.activation(out=xdbl_sb[:, ls * MMF:(ls + 1) * MMF],
                                 in_=psx, func=AF.Copy)
            nc.sync.dma_start(
                out=bcspill[:, ls * MMF:(ls + 1) * MMF],
                in_=xdbl_sb[R:R + 2 * N, ls * MMF:(ls + 1) * MMF])

        Mk_sb = bigw_pool.tile([128, ET, D], FP, tag="bigw")  # reuses hs slot
        nc.sync.dma_start(out=Mk_sb, in_=Mk.rearrange("(a p) c -> p a c", p=128))

        # ---- phase 3: the cube (exp / u / scan / C-weight / reduce) ----
        for lc in range(NLC):
            lsl = slice(lc * LC, (lc + 1) * LC)
            Bbc = bc_pool.tile([128, N, LC], FP, tag="Bbc")
            Cbc = bc_pool.tile([128, N, LC], FP, tag="Cbc")
            # partition-broadcast DMA from DRAM: every partition gets the
            # (N, LC) block of B rows (resp. C rows) for this chunk
            nc.sync.dma_start(out=Bbc, in_=bass.AP(
                tensor=bcspill.tensor, offset=lc * LC,
                ap=[[0, 128], [L, N], [1, LC]]))
            nc.sync.dma_start(out=Cbc, in_=bass.AP(
                tensor=bcspill.tensor, offset=N * L + lc * LC,
                ap=[[0, 128], [L, N], [1, LC]]))
            po = po_pool.tile([128, LC // 128, D], FP, tag="po")
            for et in range(ET):
                xl = sm_pool.tile([128, LC], FP, tag="xl")
                nc.sync.dma_start(out=xl,
                                  in_=xspill[et * 128:(et + 1) * 128, lsl])
                zl = sm_pool.tile([128, LC], FP, tag="zl")
                nc.sync.dma_start(out=zl,
                                  in_=zspill[et * 128:(et + 1) * 128, lsl])
                psd = ppool.tile([128, LC], FP, tag="mm")
                nc.tensor.matmul(psd, dtwT_sb[:, et, :], xdbl_sb[0:R, lsl],
                                 start=True, stop=True)
                # softplus(r) = ln(1 + e^r); r << 88 so e^r cannot overflow.
                # Exp and Ln share one ACT table (natural_log_exp_and_others).
                expr = sm_pool.tile([128, LC], FP, tag="v")
                nc.scalar.activation(out=expr, in_=psd, func=AF.Exp,
                                     bias=dtb_sb[:, et, :], scale=1.0)
                delta = sm_pool.tile([128, LC], FP, tag="delta")
                nc.scalar.activation(out=delta, in_=expr, func=AF.Ln, bias=1.0)
                v = sm_pool.tile([128, LC], FP, tag="v")
                nc.vector.tensor_tensor(out=v, in0=delta, in1=xl, op=OP.mult)
                H = big_pool.tile([128, N, LC], FP, tag="cube")
                for n in range(N):
                    a_n = nu_pool.tile([128, LC], FP, tag="a")
                    nc.scalar.activation(out=a_n, in_=delta, func=AF.Exp,
                                         scale=Amat_sb[:, et, n:n + 1])
                    u_n = nu_pool.tile([128, LC], FP, tag="u")
                    nc.gpsimd.tensor_tensor(out=u_n, in0=v, in1=Bbc[:, n, :],
                                            op=OP.mult)
                    init = 0.0 if lc == 0 else carry[:, et, n:n + 1]
                    nc.vector.tensor_tensor_scan(
                        out=H[:, n, :], data0=a_n, data1=u_n, initial=init,
                        op0=OP.mult, op1=OP.add)
                if lc < NLC - 1:
                    nc.vector.tensor_copy(out=carry[:, et, :],
                                          in_=H[:, :, LC - 1])
                Wt = big_pool.tile([128, N, LC], FP, tag="cube")
                nc.gpsimd.tensor_tensor(out=Wt, in0=H, in1=Cbc, op=OP.mult)
                y = sm_pool.tile([128, LC], FP, tag="y")
                nc.vector.tensor_reduce(out=y, in_=Wt.rearrange("p n t -> p t n"),
                                        axis=AX.X, op=OP.add)
                t2 = sm_pool.tile([128, LC], FP, tag="t2")
                nc.vector.scalar_tensor_tensor(out=t2, in0=xl,
                                               scalar=Dp_sb[:, et, :], in1=y,
                                               op0=OP.mult, op1=OP.add)
                yf = sm_pool.tile([128, LC], FP, tag="yf")
                nc.vector.tensor_tensor(out=yf, in0=t2, in1=zl, op=OP.mult)
                if yspill is not None:
                    nc.sync.dma_start(
                        out=yspill[et * 128:(et + 1) * 128, lsl], in_=yf)
                for lb in range(LC // 128):
                    for dh in range(D // MMF):
                        nc.tensor.matmul(
                            po[:, lb, dh * MMF:(dh + 1) * MMF],
                            yf[:, lb * 128:(lb + 1) * 128],
                            Mk_sb[:, et, dh * MMF:(dh + 1) * MMF],
                            start=(et == 0), stop=(et == ET - 1))
            for lb in range(LC // 128):
                for dh in range(D // MMF):
                    ot = smf_pool.tile([128, MMF], FP, tag="zt")
                    nc.scalar.activation(out=ot, in_=po[:, lb, dh * MMF:(dh + 1) * MMF],
                                         func=AF.Copy)
                    nc.sync.dma_start(
                        out=opart[lc * LC + lb * 128:lc * LC + (lb + 1) * 128,
                                  dh * MMF:(dh + 1) * MMF],
                        in_=ot)

        # ---- phase 4: pair AllReduce + output ----
        nc.gpsimd.collective_compute(
            "AllReduce", OP.add,
            replica_groups=[[0, 1], [2, 3], [4, 5], [6, 7]],
            ins=[opart.opt()], outs=[oshared.opt()])
        nc.sync.dma_start(out=out, in_=oshared)

    nc.compile()
    return nc


def _get_program(dbg=False):
    key = "nc_dbg" if dbg else "nc"
    if key not in _CACHE:
        _CACHE[key] = _build_program(dbg)
    return _CACHE[key]


def kernel(**inputs):
    nc = _get_program()
    f32 = lambda a: np.ascontiguousarray(np.asarray(a), dtype=np.float32)
    hs = f32(inputs["hidden_states"])            # (B, L, D)
    winT = f32(np.asarray(inputs["in_proj_w"], dtype=np.float32).T)
    xpT = f32(np.asarray(inputs["x_proj_w"], dtype=np.float32).T)
    agg_w = f32(inputs["agg_w"])
    out_w = f32(inputs["out_w"])
    conv_w = f32(inputs["conv_w"])
    conv_b = f32(inputs["conv_b"])
    dt_w = f32(inputs["dt_w"])
    dt_b = f32(inputs["dt_b"])
    A_log = f32(inputs["A_log"])
    D_param = f32(inputs["D_param"])

    Mks = [f32((out_w @ agg_w[:, k * E:(k + 1) * E]).T) for k in range(K)]
    dtwTs = [f32(dt_w[k].T) for k in range(K)]
    Amats = [f32(-np.exp(A_log[k])) for k in range(K)]

    in_maps = []
    for c in range(NCORES):
        b, k = c // 2, c % 2
        in_maps.append({
            "hsT": f32(hs[b].T),
            "winT": winT,
            "xpT": xpT,
            "dtwT": dtwTs[k],
            "dtb": f32(dt_b[k][:, None]),
            "convw": f32(conv_w[k]),
            "convb": f32(conv_b[k][:, None]),
            "Amat": Amats[k],
            "Dp": f32(D_param[k][:, None]),
            "Mk": Mks[k],
        })
    _CACHE["in_maps"] = in_maps
    res = run_bass_kernel_spmd(nc, in_maps, list(range(NCORES)))
    _CACHE["last_results"] = res.results
    out = np.empty((B, L, D), np.float32)
    for b in range(B):
        out[b] = res.results[2 * b]["out"]
    return out



# revision 3
# speedup vs baseline: 1.5748x; 1.5748x over previous
"""Trainium2 Bass kernel for the 2-module Mamba-style SSM block.

Sharding: 8 cores = 4 batches x 2 modules (core c -> batch c//2, module c%2).
Each core computes one full branch for one batch; aggregate+out_proj folded
into M_k per module; pair-wise AllReduce; host picks one core per batch.

v2: channels on partitions, fp16 cube, L chunked at LC=512.
Per chunk: in_proj (PE, f16) -> depthwise conv (DVE stt) -> x_proj (PE)
-> dt proj + softplus (PE+ACT) -> cube per channel-tile:
  a_n = exp(A_n * delta) fused on ACT (scale=A), u = v (x) B via one
  broadcast-AP f16 tensor_tensor (DVE/Pool halves), 16 scans on DVE,
  C-mult + tree-reduce split DVE/Pool, gating, then out_proj (PE, f16
  stationary yf blocks) accumulated over channel tiles in PSUM.
"""
from contextlib import ExitStack

import numpy as np

import concourse.bass as bass
import concourse.tile as tile
from concourse import bacc, mybir
from concourse.bass_utils import run_bass_kernel_spmd

FP = mybir.dt.float32
F16 = mybir.dt.float16
AX = mybir.AxisListType
OP = mybir.AluOpType
AF = mybir.ActivationFunctionType

B, L, D = 4, 2048, 1024
E, N, CW, K, R = 2048, 16, 4, 2, 64
ET = E // 128           # 16 channel tiles
DT = D // 128           # 8 d_model tiles
LC = 512                # chunk length along L
NLC = L // LC           # 4 chunks
MMF = 512               # matmul moving free size
NCORES = 8

_CACHE = {}


def _build_program():
    nc = bacc.Bacc("TRN2", target_bir_lowering=False, debug=False,
                   num_devices=NCORES)

    def din(name, shape, dt=F16):
        return nc.dram_tensor(name, list(shape), dt, kind="ExternalInput").ap()

    hsT = din("hsT", (D, L))              # hidden_states[b].T, f16
    winT = din("winT", (D, 2 * E))        # in_proj_w.T, f16
    xpT = din("xpT", (E, R + 2 * N))      # x_proj_w.T, f16
    dtwT = din("dtwT", (R, E))            # dt_w[k].T, f16
    dtb = din("dtb", (E, 1), FP)
    convw = din("convw", (E, CW), FP)
    convb = din("convb", (E, 1), FP)
    Amat = din("Amat", (E, N), FP)        # -exp(A_log[k])
    Dp = din("Dp", (E, 1), FP)
    Mk = din("Mk", (E, D))                # (out_w @ agg_w[:, k*E:(k+1)*E]).T, f16
    out = nc.dram_tensor("out", [L, D], FP, kind="ExternalOutput").ap()

    zspill = nc.dram_tensor("zspill", [E, L], F16).ap()
    bcspill = nc.dram_tensor("bcspill", [2 * N, L], F16).ap()

    with tile.TileContext(nc) as tc, ExitStack() as ctx:
        const = ctx.enter_context(tc.tile_pool(name="const", bufs=1))
        dram = ctx.enter_context(tc.tile_pool(name="dram", bufs=1, space="DRAM"))
        wpool = ctx.enter_context(tc.tile_pool(name="wpool", bufs=2))
        ch_pool = ctx.enter_context(tc.tile_pool(name="chp", bufs=1))
        hs_pool = ctx.enter_context(tc.tile_pool(name="hsp", bufs=1))
        u_pool = ctx.enter_context(tc.tile_pool(name="up", bufs=1))
        h_pool = ctx.enter_context(tc.tile_pool(name="hp", bufs=2))
        sm_pool = ctx.enter_context(tc.tile_pool(name="sm", bufs=4))
        a_pool = ctx.enter_context(tc.tile_pool(name="ap", bufs=3))
        t_pool = ctx.enter_context(tc.tile_pool(name="tp", bufs=2))
        y_pool = ctx.enter_context(tc.tile_pool(name="yp", bufs=2))
        ev_pool = ctx.enter_context(tc.tile_pool(name="ev", bufs=2))
        xd_pool = ctx.enter_context(tc.tile_pool(name="xd", bufs=2))
        pin = ctx.enter_context(tc.tile_pool(name="pin", bufs=2, space="PSUM"))
        pxp = ctx.enter_context(tc.tile_pool(name="pxp", bufs=2, space="PSUM"))
        pdt = ctx.enter_context(tc.tile_pool(name="pdt", bufs=2, space="PSUM"))
        pout = ctx.enter_context(tc.tile_pool(name="pout", bufs=2, space="PSUM"))

        opart = dram.tile([L, D], FP)
        oshared = dram.tile([L, D], FP)

        # ---- resident constants ----
        xpT_sb = const.tile([128, ET, R + 2 * N], F16)
        nc.sync.dma_start(out=xpT_sb,
                          in_=xpT.rearrange("(a p) c -> p a c", p=128))
        dtwT_sb = const.tile([R, ET, 128], F16)
        nc.sync.dma_start(out=dtwT_sb,
                          in_=dtwT.rearrange("p (a c) -> p a c", c=128))
        Amat_sb = const.tile([128, ET, N], FP)
        nc.sync.dma_start(out=Amat_sb,
                          in_=Amat.rearrange("(a p) n -> p a n", p=128))
        dtb_sb = const.tile([128, ET, 1], FP)
        nc.sync.dma_start(out=dtb_sb, in_=dtb.rearrange("(a p) o -> p a o", p=128))
        Dp_sb = const.tile([128, ET, 1], FP)
        nc.sync.dma_start(out=Dp_sb, in_=Dp.rearrange("(a p) o -> p a o", p=128))
        convw_sb = const.tile([128, ET, CW], FP)
        nc.sync.dma_start(out=convw_sb,
                          in_=convw.rearrange("(a p) c -> p a c", p=128))
        convb_sb = const.tile([128, ET, 1], FP)
        nc.sync.dma_start(out=convb_sb,
                          in_=convb.rearrange("(a p) o -> p a o", p=128))
        Mk_sb = const.tile([128, ET, D], F16)
        nc.sync.dma_start(out=Mk_sb, in_=Mk.rearrange("(a p) c -> p a c", p=128))
        carry = const.tile([128, ET, N], FP)
        halo = const.tile([128, ET, CW - 1], F16)
        nc.vector.memset(halo, 0.0)

        for lc in range(NLC):
            lsl = slice(lc * LC, (lc + 1) * LC)
            # ---- in_proj (PE): xz[:, lsl] for all 2E channels ----
            hs_sb = hs_pool.tile([128, DT, LC], F16, tag="hs")
            for dt_ in range(DT):
                nc.sync.dma_start(out=hs_sb[:, dt_, :],
                                  in_=hsT[dt_ * 128:(dt_ + 1) * 128, lsl])
            xtp = ch_pool.tile([128, ET, CW - 1 + LC], F16, tag="xtp")
            xt = ch_pool.tile([128, ET, LC], F16, tag="xt")
            for ct in range(2 * ET):
                win_ct = wpool.tile([128, DT, 128], F16, tag="win")
                nc.sync.dma_start(
                    out=win_ct,
                    in_=winT[:, ct * 128:(ct + 1) * 128].rearrange(
                        "(a p) c -> p a c", p=128))
                psums = []
                for _h in range(LC // MMF):
                    ps_in = pin.tile([128, MMF], FP, tag="mmin", name=f"psin{_h}")
                    psums.append(ps_in)
                for dt_ in range(DT):
                    for h, ps in enumerate(psums):
                        nc.tensor.matmul(ps, win_ct[:, dt_, :],
                                         hs_sb[:, dt_, h * MMF:(h + 1) * MMF],
                                         start=(dt_ == 0), stop=(dt_ == DT - 1))
                for h, ps in enumerate(psums):
                    if ct < ET:
                        nc.scalar.activation(
                            out=xtp[:, ct, CW - 1 + h * MMF:CW - 1 + (h + 1) * MMF],
                            in_=ps, func=AF.Copy)
                    else:
                        zt = ev_pool.tile([128, MMF], F16, tag="zt")
                        nc.scalar.activation(out=zt, in_=ps, func=AF.Silu)
                        nc.sync.dma_start(
                            out=zspill[(ct - ET) * 128:(ct - ET + 1) * 128,
                                       lc * LC + h * MMF:lc * LC + (h + 1) * MMF],
                            in_=zt)

            # ---- depthwise causal conv + silu (DVE + ACT) ----
            for et in range(ET):
                nc.scalar.activation(out=xtp[:, et, 0:CW - 1],
                                     in_=halo[:, et, :], func=AF.Copy)
                acc = t_pool.tile([128, LC], F16, tag="conv")
                nc.vector.tensor_scalar(out=acc, in0=xtp[:, et, 0:LC],
                                        scalar1=convw_sb[:, et, 0:1],
                                        scalar2=None, op0=OP.mult)
                for j in range(1, CW):
                    nc.vector.scalar_tensor_tensor(
                        out=acc, in0=xtp[:, et, j:j + LC],
                        scalar=convw_sb[:, et, j:j + 1], in1=acc,
                        op0=OP.mult, op1=OP.add)
                nc.scalar.activation(out=halo[:, et, :],
                                     in_=xtp[:, et, LC:LC + CW - 1], func=AF.Copy)
                nc.scalar.activation(out=xt[:, et, :], in_=acc, func=AF.Silu,
                                     bias=convb_sb[:, et, :], scale=1.0)

            # ---- x_proj (PE): x_dbl (96, LC) ----
            psx = pxp.tile([R + 2 * N, LC], FP, tag="mmxp")
            for et in range(ET):
                nc.tensor.matmul(psx, xpT_sb[:, et, :], xt[:, et, :],
                                 start=(et == 0), stop=(et == ET - 1))
            xdbl = xd_pool.tile([R + 2 * N, LC], F16, tag="xdbl")
            nc.scalar.activation(out=xdbl, in_=psx, func=AF.Copy)
            nc.sync.dma_start(out=bcspill[:, lsl], in_=xdbl[R:R + 2 * N, :])
            Bbc = ch_pool.tile([128, N, LC], F16, tag="Bbc")
            Cbc = ch_pool.tile([128, N, LC], F16, tag="Cbc")
            nc.sync.dma_start(out=Bbc, in_=bass.AP(
                tensor=bcspill.tensor, offset=lc * LC,
                ap=[[0, 128], [L, N], [1, LC]]))
            nc.sync.dma_start(out=Cbc, in_=bass.AP(
                tensor=bcspill.tensor, offset=N * L + lc * LC,
                ap=[[0, 128], [L, N], [1, LC]]))

            # ---- dt proj + cube per channel tile ----
            for et in range(ET):
                psd = pdt.tile([128, LC], FP, tag="mmdt")
                nc.tensor.matmul(psd, dtwT_sb[:, et, :], xdbl[0:R, :],
                                 start=True, stop=True)
                # softplus(r) = ln(1 + e^r); Exp and Ln share one ACT table
                expr = sm_pool.tile([128, LC], FP, tag="sm")
                nc.scalar.activation(out=expr, in_=psd, func=AF.Exp,
                                     bias=dtb_sb[:, et, :], scale=1.0)
                delta = sm_pool.tile([128, LC], FP, tag="sm")
                nc.scalar.activation(out=delta, in_=expr, func=AF.Ln, bias=1.0)
                v = t_pool.tile([128, LC], F16, tag="v")
                nc.vector.tensor_tensor(out=v, in0=delta, in1=xt[:, et, :],
                                        op=OP.mult)
                vb = v[:, :].rearrange("p (o t) -> p o t", o=1)
                uD = u_pool.tile([128, N // 2, LC], F16, tag="uD")
                uP = u_pool.tile([128, N // 2, LC], F16, tag="uP")
                nc.vector.tensor_tensor(out=uD,
                                        in0=vb.broadcast_to([128, N // 2, LC]),
                                        in1=Bbc[:, 0:N // 2, :], op=OP.mult)
                nc.gpsimd.tensor_tensor(out=uP,
                                        in0=vb.broadcast_to([128, N // 2, LC]),
                                        in1=Bbc[:, N // 2:N, :], op=OP.mult)
                H = h_pool.tile([128, N, LC], F16, tag="H")
                for n in range(N):
                    a_n = a_pool.tile([128, LC], F16, tag="a")
                    nc.scalar.activation(out=a_n, in_=delta, func=AF.Exp,
                                         scale=Amat_sb[:, et, n:n + 1])
                    un = uD[:, n, :] if n < N // 2 else uP[:, n - N // 2, :]
                    init = 0.0 if lc == 0 else carry[:, et, n:n + 1]
                    nc.vector.tensor_tensor_scan(
                        out=H[:, n, :], data0=a_n, data1=un, initial=init,
                        op0=OP.mult, op1=OP.add)
                if lc < NLC - 1:
                    nc.vector.tensor_copy(out=carry[:, et, :],
                                          in_=H[:, :, LC - 1])
                # C-mult (split DVE/Pool) + tree reduce over n
                nc.vector.tensor_tensor(out=H[:, 0:8, :], in0=H[:, 0:8, :],
                                        in1=Cbc[:, 0:8, :], op=OP.mult)
                nc.gpsimd.tensor_tensor(out=H[:, 8:16, :], in0=H[:, 8:16, :],
                                        in1=Cbc[:, 8:16, :], op=OP.mult)
                nc.gpsimd.tensor_tensor(out=H[:, 0:8, :], in0=H[:, 0:8, :],
                                        in1=H[:, 8:16, :], op=OP.add)
                nc.vector.tensor_tensor(out=H[:, 0:4, :], in0=H[:, 0:4, :],
                                        in1=H[:, 4:8, :], op=OP.add)
                nc.vector.tensor_tensor(out=H[:, 0:2, :], in0=H[:, 0:2, :],
                                        in1=H[:, 2:4, :], op=OP.add)
                y = y_pool.tile([128, LC], FP, tag="y")
                nc.vector.tensor_tensor(out=y, in0=H[:, 0, :], in1=H[:, 1, :],
                                        op=OP.add)
                zs = t_pool.tile([128, LC], F16, tag="zs")
                nc.sync.dma_start(out=zs,
                                  in_=zspill[et * 128:(et + 1) * 128, lsl])
                t2 = t_pool.tile([128, LC], F16, tag="t2")
                nc.vector.scalar_tensor_tensor(out=t2, in0=xt[:, et, :],
                                               scalar=Dp_sb[:, et, :], in1=y,
                                               op0=OP.mult, op1=OP.add)
                nc.vector.tensor_tensor(out=xt[:, et, :], in0=t2, in1=zs,
                                        op=OP.mult)

            # ---- out_proj (PE): yf^T @ Mk, accumulated over et ----
            for tau in range(LC // 128):
                for dh in range(D // MMF):
                    po = pout.tile([128, MMF], FP, tag="mmo")
                    for et in range(ET):
                        nc.tensor.matmul(
                            po, xt[:, et, tau * 128:(tau + 1) * 128],
                            Mk_sb[:, et, dh * MMF:(dh + 1) * MMF],
                            start=(et == 0), stop=(et == ET - 1))
                    osb = ev_pool.tile([128, MMF], FP, tag="osb")
                    nc.scalar.activation(out=osb, in_=po, func=AF.Copy)
                    nc.sync.dma_start(
                        out=opart[lc * LC + tau * 128:lc * LC + (tau + 1) * 128,
                                  dh * MMF:(dh + 1) * MMF],
                        in_=osb)

        # ---- pair AllReduce + output ----
        nc.gpsimd.collective_compute(
            "AllReduce", OP.add,
            replica_groups=[[0, 1], [2, 3], [4, 5], [6, 7]],
            ins=[opart.opt()], outs=[oshared.opt()])
        nc.sync.dma_start(out=out, in_=oshared)

    nc.compile()
    return nc


def _get_program():
    if "nc" not in _CACHE:
        _CACHE["nc"] = _build_program()
    return _CACHE["nc"]


def kernel(**inputs):
    nc = _get_program()
    f32 = lambda a: np.ascontiguousarray(np.asarray(a), dtype=np.float32)
    f16 = lambda a: np.ascontiguousarray(np.asarray(a, dtype=np.float32),
                                         dtype=np.float16)
    hs = np.asarray(inputs["hidden_states"], dtype=np.float32)   # (B, L, D)
    winT = f16(np.asarray(inputs["in_proj_w"], dtype=np.float32).T)
    xpT = f16(np.asarray(inputs["x_proj_w"], dtype=np.float32).T)
    agg_w = f32(inputs["agg_w"])
    out_w = f32(inputs["out_w"])
    conv_w = f32(inputs["conv_w"])
    conv_b = f32(inputs["conv_b"])
    dt_w = f32(inputs["dt_w"])
    dt_b = f32(inputs["dt_b"])
    A_log = f32(inputs["A_log"])
    D_param = f32(inputs["D_param"])

    Mks = [f16((out_w @ agg_w[:, k * E:(k + 1) * E]).T) for k in range(K)]
    dtwTs = [f16(dt_w[k].T) for k in range(K)]
    Amats = [f32(-np.exp(A_log[k])) for k in range(K)]

    in_maps = []
    for c in range(NCORES):
        b, k = c // 2, c % 2
        in_maps.append({
            "hsT": f16(hs[b].T),
            "winT": winT,
            "xpT": xpT,
            "dtwT": dtwTs[k],
            "dtb": f32(dt_b[k][:, None]),
            "convw": f32(conv_w[k]),
            "convb": f32(conv_b[k][:, None]),
            "Amat": Amats[k],
            "Dp": f32(D_param[k][:, None]),
            "Mk": Mks[k],
        })
    _CACHE["in_maps"] = in_maps
    res = run_bass_kernel_spmd(nc, in_maps, list(range(NCORES)))
    _CACHE["last_results"] = res.results
    out = np.empty((B, L, D), np.float32)
    for b in range(B):
        out[b] = res.results[2 * b]["out"]
    return out
